# revision 1
# baseline (speedup 1.0000x reference)
"""Trainium2 Bass kernel for an attention-LSTM caption decoder.

Math notes (verified against the reference on CPU):
  - num_pixels == 1 makes the softmax attention exactly a no-op: alpha == 1.0,
    so awe = sigmoid(h @ W_beta) * features. W_enc/W_dec/W_full are unused.
  - Masked (b, t) rows (t >= len[b]) never re-activate and never influence
    active rows, so h/c freezing can be dropped; only output masking matters.
    We compute/emit only the active rows (lengths are sorted descending, so
    the active rows at step t are a prefix of the batch).
  - All biases in setup_inputs() are zero; detected host-side, bias work is
    compiled out (fallback paths are kept for nonzero biases).

Distribution (8 cores): recurrence replicated on every core; fc weight and
the output vocab dim sharded 8-way (tensor parallel). The embedding table
(bf16) is resident in each core's HBM.

Schedule highlights vs the v0 kernel:
  - The embedding gather uses dma_gather(transpose=True) on a bf16 table,
    landing rows directly in the transposed [128, kc, row] layout; a zero
    row appended to each table half turns the lo/hi split into a plain add
    (no select mask). 4 swdge queues, gathers issued before the big weight
    DMAs so they overlap.
  - E = emb @ W_ih_emb.T is kept in SBUF (no DRAM staging) and its blocks
    are computed interleaved with the first recurrence steps.
  - The LSTM gate PSUM is split into four independent 512-wide tiles so the
    pointwise chain on chunk f starts while chunks i/o/g are still being
    accumulated on the PE (awe matmuls are emitted f,i,o,g).
  - Vocab-projection matmuls fill the PE idle window during the pointwise
    chain; their PSUM->SBUF copies alternate between ScalarE and VectorE.
"""

import numpy as np

from concourse import bacc, bass, library_config, mybir, tile
from concourse.bass_utils import run_bass_kernel_spmd

F32 = mybir.dt.float32
BF16 = mybir.dt.bfloat16
I16 = mybir.dt.int16

B = 64
H = 512
T = 20
V = 50257
NCORES = 8
VS = 6284            # per-core vocab shard (8 * 6284 = 50272 >= 50257)
VPAD = NCORES * VS
KC_H = H // 128      # 4 k-chunks per 512-wide contraction
GATE_N = 4 * H       # 2048
SPLIT = 32000        # embedding table split (int16 index range)
NHI = V - SPLIT      # 18257 hi rows
Z_HI = NHI           # local index of the zero row in the hi half

N_GATHER = (T - 1) * B           # 1216 gathered rows, j = (t-1)*64 + b
EMB_BLOCKS = 10                  # E blocks of 128 rows (1280 incl features)
FEAT_OFF = N_GATHER              # features rows live at 1216..1279 (block 9 hi)
# 128-aligned groups: g0 small so steps 1-2 unblock as early as possible
GATHER_GROUPS = [(0, 128), (128, 512), (512, 896), (896, 1280)]
VCHUNKS = [(i * 512, min(512, VS - i * 512)) for i in range(13)]
VOCAB_CAP = 5                    # max vocab items interleaved per step

# idxpack layout: per group, int16 indices then the parity mask (bf16 bits)
_IPK = []
_IPK_W = 0
for _a, _b in GATHER_GROUPS:
    _n = _b - _a
    _IPK.append((_IPK_W, _n // 16, _n))
    _IPK_W += _n // 16 + _n


def _pack_k(w):
    """[K, N] -> [128, K//128, N] with the contraction dim on partitions."""
    k, n = w.shape
    assert k % 128 == 0
    return np.ascontiguousarray(w.reshape(k // 128, 128, n).transpose(1, 0, 2))


def _pack_idx(a):
    """(n,) int16 -> [128, n//16]; j = s*16 + p wrapping, replicated into each
    of the 8 GPSIMD Q7 cores' 16-partition groups."""
    n = a.shape[0]
    assert n % 16 == 0
    out = np.zeros((128, n // 16), np.int16)
    for c in range(8):
        out[16 * c : 16 * c + 16, :] = a.reshape(n // 16, 16).T
    return out


def _host_prep(inputs):
    import ml_dtypes

    bf16 = ml_dtypes.bfloat16
    f32 = np.float32
    feats = np.asarray(inputs["features"], f32)
    caps = np.asarray(inputs["captions"]).astype(np.int64)
    lens = np.asarray(inputs["lengths"]).reshape(-1).astype(np.int64)
    table = np.asarray(inputs["embed_table"], f32)

    W_ih = np.asarray(inputs["W_ih"], f32)
    W_hh = np.asarray(inputs["W_hh"], f32)
    b_ih = np.asarray(inputs["b_ih"], f32)
    b_hh = np.asarray(inputs["b_hh"], f32)
    b_beta = np.asarray(inputs["b_beta"], f32)
    b_fc = np.asarray(inputs["b_fc"], f32)
    b_hinit = np.asarray(inputs["b_hinit"], f32)
    b_cinit = np.asarray(inputs["b_cinit"], f32)

    # ragged-batch packing (lengths sorted descending by construction)
    b_t = [int((lens > t).sum()) for t in range(T)]
    off = np.concatenate([[0], np.cumsum(b_t)]).astype(np.int64)
    p_total = int(off[-1])
    p_pad = ((p_total + 127) // 128) * 128

    # paired-row table: two vocab rows per gathered "super-row", so the
    # super-row index (i // 2, max 25128) fits int16 and one descriptor
    # fetches 2 KiB. A parity mask selects the right half after the gather.
    VP = V + (V % 2)
    tpad = np.zeros((VP, H), f32)
    tpad[:V] = table
    tableg2 = np.ascontiguousarray(tpad.reshape(VP // 2, 2 * H)).astype(bf16)

    # gather indices, t-major (t=1..19); 64 pad rows -> index 0
    idx_flat = np.zeros(EMB_BLOCKS * 128, np.int64)
    idx_flat[:N_GATHER] = caps.T.reshape(-1)
    idx_sup = (idx_flat // 2).astype(np.int16)
    parity = (idx_flat % 2).astype(f32)

    w2emb = W_ih.T[:H]                      # [512, 2048] emb input rows
    w2ah = np.vstack([W_ih.T[H:], W_hh.T])  # [1024, 2048] awe+h input rows

    b2 = b_ih + b_hh
    has_b2 = bool(np.any(b2))
    has_bbeta = bool(np.any(b_beta))
    has_bfc = bool(np.any(b_fc))
    has_binit = bool(np.any(b_hinit)) or bool(np.any(b_cinit))

    common = {
        "tableg2": tableg2,
        "featT": _pack_k(feats.T.astype(f32)).astype(bf16),
        "w2e": _pack_k(w2emb).astype(bf16),
        "w2ah": _pack_k(w2ah).astype(bf16),
        "wbeta": _pack_k(np.asarray(inputs["W_beta"], f32)).astype(bf16),
    }
    # small constants in ONE tensor (each dma_start costs ~1us of sequencer
    # descriptor-writing; 14 small DMAs serialized the whole prep head):
    # cols 0:128 ident | 128:192 identhi | 192:448 unused | 448:960 features
    cpk = np.zeros((128, 960), bf16)
    cpk[:, 0:128] = np.eye(128, dtype=f32).astype(bf16)
    for i in range(64):
        cpk[64 + i, 128 + i] = f32(1.0)
    cpk[0:64, 448:960] = feats.astype(bf16)
    common["constpack"] = cpk
    whc = np.zeros((128, KC_H, 2 * H), bf16)
    whc[:, :, 0:H] = _pack_k(np.asarray(inputs["W_hinit"], f32)).astype(bf16)
    whc[:, :, H : 2 * H] = _pack_k(np.asarray(inputs["W_cinit"], f32)).astype(bf16)
    common["whcpack"] = whc
    ipk = np.zeros((128, _IPK_W), np.int16)
    for g, (a, b) in enumerate(GATHER_GROUPS):
        o, w, n = _IPK[g]
        ipk[:, o : o + w] = _pack_idx(idx_sup[a:b])
        mk = np.ascontiguousarray(
            np.tile(parity[a:b][None, :], (128, 1))).astype(bf16)
        ipk[:, o + w : o + w + n] = mk.view(np.int16)
    common["idxpack"] = ipk
    if has_b2:
        common["b2rep"] = np.ascontiguousarray(
            np.tile(b2[None, :], (128, 1)).astype(f32)
        )
    if has_bbeta or has_binit:
        common["ones"] = np.ones((1, B), bf16)
    if has_bbeta:
        common["bbetarow"] = b_beta.reshape(1, H).astype(bf16)
    if has_binit:
        common["bhinitT"] = np.ascontiguousarray(
            b_hinit.reshape(KC_H, 128).T.astype(f32)
        )
        common["bcinitrow"] = b_cinit.reshape(1, H).astype(bf16)

    W_fc = np.asarray(inputs["W_fc"], f32)
    wfc_pad = np.zeros((H, VPAD), f32)
    wfc_pad[:, :V] = W_fc
    bfc_pad = np.zeros(VPAD, f32)
    bfc_pad[:V] = b_fc

    in_maps = []
    for k in range(NCORES):
        m = dict(common)
        m["wfc"] = _pack_k(wfc_pad[:, k * VS : (k + 1) * VS]).astype(bf16)
        if has_bfc:
            m["bfcrep"] = np.ascontiguousarray(
                np.tile(bfc_pad[k * VS : (k + 1) * VS][None, :], (128, 1))
            ).astype(f32)
        in_maps.append(m)

    meta = {
        "b_t": b_t, "off": off, "p_total": p_total, "p_pad": p_pad,
        "has_b2": has_b2, "has_bbeta": has_bbeta, "has_bfc": has_bfc,
        "has_binit": has_binit,
    }
    return in_maps, meta


def build_program(meta):
    """Build the (SPMD-identical) Bass program. Per-core differences are data
    only (wfc shards)."""
    b_t = meta["b_t"]
    off = [int(x) for x in meta["off"]]
    p_total = meta["p_total"]
    p_pad = meta["p_pad"]
    mv = p_pad // 128
    has_b2 = meta["has_b2"]
    has_bbeta = meta["has_bbeta"]
    has_bfc = meta["has_bfc"]
    has_binit = meta["has_binit"]

    nc = bacc.Bacc(num_swdge_queues=1)

    tableg2_d = nc.declare_dram_parameter(
        "tableg2", [(V + (V % 2)) // 2, 2 * H], BF16, isOutput=False)
    featT_d = nc.declare_dram_parameter("featT", [128, KC_H, B], BF16, isOutput=False)
    w2e_d = nc.declare_dram_parameter("w2e", [128, KC_H, GATE_N], BF16, isOutput=False)
    w2ah_d = nc.declare_dram_parameter("w2ah", [128, 8, GATE_N], BF16, isOutput=False)
    wbeta_d = nc.declare_dram_parameter("wbeta", [128, KC_H, H], BF16, isOutput=False)
    constpack_d = nc.declare_dram_parameter("constpack", [128, 960], BF16, isOutput=False)
    whcpack_d = nc.declare_dram_parameter("whcpack", [128, KC_H, 2 * H], BF16, isOutput=False)
    idxpack_d = nc.declare_dram_parameter("idxpack", [128, _IPK_W], I16, isOutput=False)
    wfc_d = nc.declare_dram_parameter("wfc", [128, KC_H, VS], BF16, isOutput=False)
    if has_b2:
        b2rep_d = nc.declare_dram_parameter("b2rep", [128, GATE_N], F32, isOutput=False)
    if has_bbeta or has_binit:
        ones_d = nc.declare_dram_parameter("ones", [1, B], BF16, isOutput=False)
    if has_bbeta:
        bbetarow_d = nc.declare_dram_parameter("bbetarow", [1, H], BF16, isOutput=False)
    if has_binit:
        bhinitT_d = nc.declare_dram_parameter("bhinitT", [128, KC_H], F32, isOutput=False)
        bcinitrow_d = nc.declare_dram_parameter("bcinitrow", [1, H], BF16, isOutput=False)
    if has_bfc:
        bfcrep_d = nc.declare_dram_parameter("bfcrep", [128, VS], F32, isOutput=False)
    out_d = nc.declare_dram_parameter("out", [p_pad, VS], F32, isOutput=True)

    def mm(out, lhsT, rhs, start, stop):
        nc.tensor.matmul(out, lhsT, rhs, start=start, stop=stop)

    # vocab work item (mc, vo, vw) is runnable once all hall rows of block mc
    # are written, i.e. after step mc_ready[mc]'s h transposes.
    mc_ready = []
    for mc in range(mv):
        need = (mc + 1) * 128
        r = T - 1
        for t in range(T):
            if off[t + 1] >= need:
                r = t
                break
        mc_ready.append(r)
    vqueue = [(mc, vo, vw) for mc in range(mv) for (vo, vw) in VCHUNKS]

    # per-step hall write segments: (mc, local_lo, src_lo, n)
    hall_segs = []
    for t in range(T):
        segs = []
        lo, n = off[t], b_t[t]
        while n > 0:
            mc = lo // 128
            ll = lo % 128
            take = min(128 - ll, n)
            segs.append((mc, ll, lo - off[t], take))
            lo += take
            n -= take
        hall_segs.append(segs)

    SIG = mybir.ActivationFunctionType.Sigmoid
    TANH = mybir.ActivationFunctionType.Tanh

    with tile.TileContext(nc) as tc:
        nc.gpsimd.load_library(library_config.mlp)
        with (
            tc.tile_pool(name="const", bufs=1) as constp,
            tc.tile_pool(name="res", bufs=1) as resp,
            tc.tile_pool(name="state", bufs=1) as statep,
            tc.tile_pool(name="step", bufs=1) as sp,
            tc.tile_pool(name="stage", bufs=3) as stp,
            tc.tile_pool(name="gpsum", bufs=1, space="PSUM") as gps,
            tc.tile_pool(name="fill", bufs=2, space="PSUM") as fillp,
            tc.tile_pool(name="tpsum", bufs=2, space="PSUM") as tps,
        ):
            # --- tiny constants first so their DMAs head the queues (featT
            # leads: h0/c0 and the features E block are the first compute) ---
            featT = constp.tile([128, KC_H, B], BF16)
            nc.sync.dma_start(featT[:], featT_d[:])
            # h0/c0 weights second: later sync DMAs get starved once the big
            # weight/gather descriptors flood the rings (whc measured 42us
            # when pushed 4th), and the first PE matmul waits on this
            whc = constp.tile([128, KC_H, 2 * H], BF16, name="whc")
            nc.sync.dma_start(whc[:], whcpack_d[:])
            cp = constp.tile([128, 960], BF16, name="cp")
            nc.sync.dma_start(cp[:], constpack_d[:])
            ipk = constp.tile([128, _IPK_W], I16, name="ipk")
            nc.sync.dma_start(ipk[:], idxpack_d[:])
            if has_bbeta or has_binit:
                ones = constp.tile([1, B], BF16)
                nc.sync.dma_start(ones[:], ones_d[:])
            if has_bbeta:
                bbetarow = constp.tile([1, H], BF16)
                nc.sync.dma_start(bbetarow[:], bbetarow_d[:])

            # --- persistent state (DMAs started after the gathers) ---
            w2ah = resp.tile([128, 8, GATE_N], BF16)
            wbeta = resp.tile([128, KC_H, H], BF16)
            e_sb = resp.tile([128, EMB_BLOCKS, GATE_N], BF16, name="E")
            hall_t = [
                resp.tile([128, KC_H, 128], BF16, tag=f"hall{mc}", name=f"hall{mc}")
                for mc in range(mv)
            ]
            if p_pad > p_total:
                mc = p_total // 128
                nc.vector.memset(hall_t[mc][:, :, p_total % 128 :], 0.0)
                for m2 in range(mc + 1, mv):
                    nc.vector.memset(hall_t[m2][:], 0.0)
            if has_b2:
                b2rep = resp.tile([128, GATE_N], F32)
            if has_bfc:
                bfcrep = resp.tile([128, VS], F32)

            wfc = resp.tile([128, KC_H, VS], BF16)

            hT = statep.tile([128, KC_H, B], BF16)
            aweT = statep.tile([128, KC_H, B], BF16)
            c_st = statep.tile([B, H], F32)

            # gate PSUM: four independent 512-wide chunks (i, f, g, o)
            g_ps = [gps.tile([B, H], F32, tag=f"g{x}", name=f"g{x}")
                    for x in range(4)]

            # =============== helpers ===============
            vstate = {"q": 0, "alt": 0}

            def vocab_item():
                mc, vo, vw = vqueue[vstate["q"]]
                vstate["q"] += 1
                pv = fillp.tile([128, H], F32, tag="fill")
                for kc in range(KC_H):
                    mm(pv[:, :vw], hall_t[mc][:, kc, :], wfc[:, kc, vo : vo + vw],
                       start=(kc == 0), stop=(kc == KC_H - 1))
                st = stp.tile([128, H], F32, tag="st")
                if has_bfc:
                    nc.vector.tensor_add(st[:, :vw], pv[:, :vw],
                                         bfcrep[:, vo : vo + vw])
                elif vstate["alt"] == 0:
                    nc.scalar.copy(st[:, :vw], pv[:, :vw])
                else:
                    nc.vector.tensor_copy(st[:, :vw], pv[:, :vw])
                vstate["alt"] ^= 1
                nc.sync.dma_start(
                    out_d[mc * 128 : (mc + 1) * 128, vo : vo + vw], st[:, :vw]
                )

            def emit_vocab(t):
                # late steps are supply-gated (blocks 4-5 only complete at the
                # end), so uncap them to shorten the post-loop drain
                cap = VOCAB_CAP if t < 14 else 16
                n = 0
                while (vstate["q"] < len(vqueue) and n < cap
                       and mc_ready[vqueue[vstate["q"]][0]] < t):
                    vocab_item()
                    n += 1

            ep_alt = [0]

            def e_pre(mb, lhsT_tile, lhsT_col0, half=None):
                """E block mb from transposed emb rows [128, KC_H, *] at
                lhsT_tile[:, kc, lhsT_col0:+w]. half: None=128 rows,
                'lo'/'hi' = 64-row half blocks."""
                r0, rn = (0, 128) if half is None else ((0, 64) if half == "lo" else (64, 128))
                w = rn - r0
                for nc4 in range(4):
                    pse = fillp.tile([128, H], F32, tag="fill")
                    for kc in range(KC_H):
                        mm(pse[r0:rn, :],
                           lhsT_tile[:, kc, lhsT_col0 + r0 : lhsT_col0 + r0 + w],
                           w2e[:, kc, nc4 * 512 : (nc4 + 1) * 512],
                           start=(kc == 0), stop=(kc == KC_H - 1))
                    dst = e_sb[r0:rn, mb, nc4 * 512 : (nc4 + 1) * 512]
                    if has_b2:
                        nc.vector.tensor_add(
                            dst, pse[r0:rn, :],
                            b2rep[r0:rn, nc4 * 512 : (nc4 + 1) * 512])
                    elif ep_alt[0] == 0:
                        nc.scalar.copy(dst, pse[r0:rn, :])
                    else:
                        nc.vector.tensor_copy(dst, pse[r0:rn, :])
                    ep_alt[0] ^= 1

            def e_load(t):
                """Start the gate accumulation groups with E_t (+ implicitly
                the biases, folded into E)."""
                m0 = FEAT_OFF if t == 0 else (t - 1) * B
                p0, gslc = m0 % 128, m0 // 128
                idx = cp[0:64, 0:64] if p0 == 0 else cp[64:128, 128:192]
                for x in range(4):
                    mm(g_ps[x][:], idx, e_sb[p0 : p0 + B, gslc, x * 512 : (x + 1) * 512],
                       start=True, stop=False)

            def h_trans(t, h_new):
                """Transpose h_new (= h_{t+1}, preds source of step t) into hT
                and scatter its active prefix into the hall tiles."""
                for kc in range(KC_H):
                    tp = tps.tile([128, B], BF16, tag="tp")
                    nc.tensor.transpose(
                        tp[:, 0:B], h_new[:, kc * 128 : (kc + 1) * 128],
                        cp[0:B, 0:B])
                    nc.vector.tensor_copy(hT[:, kc, :], tp[:])
                    for (mc, ll, sl, seg_n) in hall_segs[t]:
                        nc.vector.tensor_copy(
                            hall_t[mc][:, kc, ll : ll + seg_n],
                            tp[:, sl : sl + seg_n])

            def beta_mms():
                """beta = h @ W_beta [+ b_beta] into PSUM; returns the tile."""
                betaps = fillp.tile([128, H], F32, tag="fill")
                for kc in range(KC_H):
                    mm(betaps[0:B, :], hT[:, kc, :], wbeta[:, kc, :],
                       start=(kc == 0), stop=(kc == KC_H - 1) and not has_bbeta)
                if has_bbeta:
                    mm(betaps[0:B, :], ones[:], bbetarow[:], start=False, stop=True)
                return betaps

            def h_part():
                """h contribution to the gates (E already loaded)."""
                for kc in range(KC_H):
                    for x in range(4):
                        mm(g_ps[x][:], hT[:, kc, :],
                           w2ah[:, 4 + kc, x * 512 : (x + 1) * 512],
                           start=False, stop=False)

            def sig_awe(betaps):
                """awe = sigmoid(beta) * features, transposed into aweT. The
                ACT/DVE work overlaps the h_part matmuls on the PE."""
                sigb = sp.tile([B, H], BF16, tag="sigb")
                nc.scalar.activation(sigb[:], betaps[0:B, :], SIG)
                aweb = sp.tile([B, H], BF16, tag="aweb")
                nc.vector.tensor_mul(aweb[:], sigb[:], cp[0:64, 448:960])
                for kc in range(KC_H):
                    tp = tps.tile([128, B], BF16, tag="tp")
                    nc.tensor.transpose(
                        tp[:, 0:B], aweb[:, kc * 128 : (kc + 1) * 128],
                        cp[0:B, 0:B])
                    nc.vector.tensor_copy(aweT[:, kc, :], tp[:])

            def awe_and_pointwise(t):
                """awe gate matmuls + LSTM pointwise; returns h_new tile."""
                # awe contribution, chunk-major f,i,o,g so f completes first
                for x in (1, 0, 3, 2):
                    for kc in range(KC_H):
                        mm(g_ps[x][:], aweT[:, kc, :],
                           w2ah[:, kc, x * 512 : (x + 1) * 512],
                           start=False, stop=(kc == KC_H - 1))
                # pointwise (PyTorch gate order i, f, g, o)
                sig_f = sp.tile([B, H], F32, tag="sig_f")
                nc.scalar.activation(sig_f[:], g_ps[1][:], SIG)
                sig_i = sp.tile([B, H], F32, tag="sig_i")
                nc.scalar.activation(sig_i[:], g_ps[0][:], SIG)
                sig_o = sp.tile([B, H], BF16, tag="sig_o")
                nc.scalar.activation(sig_o[:], g_ps[3][:], SIG)
                tanh_g = sp.tile([B, H], F32, tag="tanh_g")
                nc.scalar.activation(tanh_g[:], g_ps[2][:], TANH)
                nc.vector.tensor_mul(c_st[:], c_st[:], sig_f[:])
                t2 = sp.tile([B, H], F32, tag="t2")
                nc.vector.tensor_mul(t2[:], sig_i[:], tanh_g[:])
                nc.vector.tensor_add(c_st[:], c_st[:], t2[:])
                tanh_c = sp.tile([B, H], BF16, tag="tanh_c")
                nc.scalar.activation(tanh_c[:], c_st[:], TANH)
                h_new = sp.tile([B, H], BF16, tag="h_new")
                nc.vector.tensor_mul(h_new[:], sig_o[:], tanh_c[:])
                return h_new

            # ================= prep phase =================
            with (
                tc.tile_pool(name="prew", bufs=1) as prew,
                tc.tile_pool(name="gath", bufs=2) as gp,
            ):
                glo = [None] * 4
                maskt = [None] * 4

                def gather(g):
                    a, b = GATHER_GROUPS[g]
                    n = b - a
                    o, w, _ = _IPK[g]
                    maskt[g] = ipk[:, o + w : o + w + n].bitcast(BF16)
                    glo[g] = gp.tile([128, 2 * KC_H, n], BF16, tag=f"graw{g}",
                                     bufs=1, name=f"graw{g}")
                    nc.gpsimd.dma_gather(
                        glo[g][:], tableg2_d[:],
                        ipk[:, o : o + w], num_idxs=n, num_idxs_reg=n,
                        elem_size=2 * H, transpose=True, queue_num=0,
                    )

                def blend(g):
                    a, b = GATHER_GROUPS[g]
                    n = b - a
                    # select even/odd row per column: raw[c] += parity*(odd-even)
                    for c in range(KC_H):
                        d = gp.tile([128, 384], BF16, tag="gd", bufs=2,
                                    name="gd")
                        nc.vector.tensor_sub(
                            d[:, :n], glo[g][:, KC_H + c, :], glo[g][:, c, :])
                        nc.vector.tensor_mul(d[:, :n], d[:, :n], maskt[g])
                        nc.vector.tensor_add(
                            glo[g][:, c, :], glo[g][:, c, :], d[:, :n])

                # all gathers upfront: the scheduler reorders per-engine
                # anyway, and early emission starts their DMAs soonest
                for g in range(4):
                    gather(g)

                # (whc DMA already issued at the head of the sync queue)

                # warm the sigmoid/tanh ACT table during the DMA wait
                warm = sp.tile([1, 2], F32, tag="warm")
                nc.scalar.activation(warm[:], cp[0:1, 0:2], SIG)

                # weight DMAs, ordered by first use
                nc.scalar.dma_start(wbeta[:], wbeta_d[:])
                w2e = prew.tile([128, KC_H, GATE_N], BF16, bufs=1)
                nc.scalar.dma_start(w2e[:], w2e_d[:])
                nc.scalar.dma_start(w2ah[:], w2ah_d[:])
                if has_b2:
                    nc.scalar.dma_start(b2rep[:], b2rep_d[:])
                if has_bfc:
                    nc.scalar.dma_start(bfcrep[:], bfcrep_d[:])

                # h0 (transposed directly) and c0
                if has_binit:
                    bhinitT = prew.tile([128, KC_H], F32)
                    nc.sync.dma_start(bhinitT[:], bhinitT_d[:])
                    bcinitrow = prew.tile([1, H], BF16)
                    nc.sync.dma_start(bcinitrow[:], bcinitrow_d[:])
                for jb in range(KC_H):
                    hps = fillp.tile([128, H], F32, tag="fill")
                    for kc in range(KC_H):
                        mm(hps[:, 0:B], whc[:, kc, jb * 128 : (jb + 1) * 128],
                           featT[:, kc, :], start=(kc == 0), stop=(kc == KC_H - 1))
                    if has_binit:
                        nc.scalar.activation(
                            hT[:, jb, :], hps[:, 0:B],
                            mybir.ActivationFunctionType.Identity,
                            bias=bhinitT[:, jb : jb + 1])
                    else:
                        nc.vector.tensor_copy(hT[:, jb, :], hps[:, 0:B])
                cps = fillp.tile([128, H], F32, tag="fill")
                for kc in range(KC_H):
                    mm(cps[0:B, :], featT[:, kc, :], whc[:, kc, H : 2 * H],
                       start=(kc == 0), stop=(kc == KC_H - 1) and not has_binit)
                if has_binit:
                    mm(cps[0:B, :], ones[:], bcinitrow[:], start=False, stop=True)
                nc.vector.tensor_copy(c_st[:], cps[0:B, :])

                # E for t=0 (features block = hi half of block 9; featT holds
                # exactly those 64 rows, hence the -64 column bias)
                e_pre(EMB_BLOCKS - 1, featT, -64, half="hi")

                # wfc DMA after the early weights (first vocab item is t>=3)
                nc.scalar.dma_start(wfc[:], wfc_d[:])

                # ---- step 0 ----
                e_load(0)
                bps = beta_mms()
                h_part()
                sig_awe(bps)
                h_new = awe_and_pointwise(0)

                # ---- steps 1..8 with interleaved gather/E work ----
                def step(t):
                    e_load(t)
                    h_trans(t - 1, h_new)
                    bps = beta_mms()
                    h_part()
                    sig_awe(bps)
                    hn = awe_and_pointwise(t)
                    emit_vocab(t)
                    return hn

                # E-block needs: step t reads block (t-1)//2
                blend(0)
                e_pre(0, glo[0], 0)
                h_new = step(1)
                h_new = step(2)
                blend(1)
                for mb in (1, 2, 3):
                    e_pre(mb, glo[1], (mb - 1) * 128)
                h_new = step(3)
                h_new = step(4)
                blend(2)
                for mb in (4, 5, 6):
                    e_pre(mb, glo[2], (mb - 4) * 128)
                h_new = step(5)
                h_new = step(6)
                blend(3)
                for mb in (7, 8):
                    e_pre(mb, glo[3], (mb - 7) * 128)
                e_pre(EMB_BLOCKS - 1, glo[3], 256, half="lo")
                h_new = step(7)
                h_new = step(8)

            # ============ steady recurrence ============
            for t in range(9, T):
                h_new = step(t)

            h_trans(T - 1, h_new)
            while vstate["q"] < len(vqueue):
                vocab_item()

    nc.finalize()
    return nc


def kernel(**inputs):
    in_maps, meta = _host_prep(inputs)
    nc = build_program(meta)
    res = run_bass_kernel_spmd(nc, in_maps, core_ids=list(range(NCORES)))
    results = res.results

    b_t = meta["b_t"]
    off = meta["off"]
    full = np.zeros((B, T, VPAD), np.float32)
    for k in range(NCORES):
        o = np.asarray(results[k]["out"])
        for t in range(T):
            bt = b_t[t]
            if bt:
                full[:bt, t, k * VS : (k + 1) * VS] = o[off[t] : off[t] + bt]
    return full[:, :, :V]



# revision 11
# speedup vs baseline: 1.5239x; 1.5239x over previous
"""Trainium2 Bass kernel for an attention-LSTM caption decoder (v2).

Math notes (verified against the reference):
  - num_pixels == 1 makes softmax attention a no-op: alpha == 1.0, so
    awe = sigmoid(h @ W_beta) * features. W_enc/W_dec/W_full unused.
  - Masked (b, t) rows never re-activate and never influence active rows, so
    h/c freezing is dropped; only output masking matters (lengths sorted
    descending -> active rows at step t are a prefix).
  - All biases in setup_inputs() are zero; detected host-side and compiled out
    (cheap fallbacks kept).

v2 design (vs v1 at ~411us):
  - Column tiling (128x64 mode): every M=64 recurrence matmul is emitted as a
    T0/T1 pair writing PSUM partitions 0-63 / 64-127 concurrently (~1.7x
    measured on HW for N=512 pairs).
  - Folded-H layout: batch lives twice on the partition axis
    (p = b + 64*s, s = H-half), so pointwise ACT/DVE ops run on all 128
    partitions at FD=256/512 instead of 64 partitions at FD=512/1024.
    Gate PSUM = 2 tiles [128,512]: (f,i) -> one sigmoid ACT covers both.
  - E rows ragged-packed to active (t,b) pairs only: E row index == hall row
    index == off[t]+b. ~6 blocks instead of 10.
  - bf16 output staging + DMA (halves write traffic); host converts to f32.
  - Weight DMAs split/ordered by first use.

Distribution (8 cores): recurrence replicated; fc weight + vocab dim sharded
8-way (tensor parallel).
"""

import numpy as np

from concourse import bacc, bass, library_config, mybir, tile
from concourse.bass_utils import run_bass_kernel_spmd

F32 = mybir.dt.float32
BF16 = mybir.dt.bfloat16
I16 = mybir.dt.int16

B = 64
H = 512
T = 20
V = 50257
NCORES = 8
VS = 6284            # per-core vocab shard (8 * 6284 = 50272 >= 50257)
VPAD = NCORES * VS
KC = 4               # k-chunks per 512-wide contraction
GATE_N = 4 * H       # 2048

VCHUNKS = [(i * 512, min(512, VS - i * 512)) for i in range(13)]

# folded gate-column order: psum tile_fi = [f|i], tile_go = [g|o]; T0 gets the
# lo H-half columns, T1 the hi half.  PyTorch gate order i,f,g,o.
_CO = np.r_[512:768, 0:256,          # cb0: f-lo | i-lo   (fi tile, T0)
            768:1024, 256:512,       # cb1: f-hi | i-hi   (fi tile, T1)
            1024:1280, 1536:1792,    # cb2: g-lo | o-lo   (go tile, T0)
            1280:1536, 1792:2048]    # cb3: g-hi | o-hi   (go tile, T1)


def _pack_k(w):
    """[K, N] -> [128, K//128, N] with the contraction dim on partitions."""
    k, n = w.shape
    assert k % 128 == 0
    return np.ascontiguousarray(w.reshape(k // 128, 128, n).transpose(1, 0, 2))


def _pack_idx(a):
    """(n,) int16 -> [128, n//16] replicated into each of the 8 GPSIMD Q7
    cores' 16-partition groups."""
    n = a.shape[0]
    assert n % 16 == 0
    out = np.zeros((128, n // 16), np.int16)
    for c in range(8):
        out[16 * c : 16 * c + 16, :] = a.reshape(n // 16, 16).T
    return out


def _fold(x):
    """[B, H] -> [128, 256] folded: out[b + 64*s, n] = x[b, 256*s + n]."""
    b, h = x.shape
    assert b == 64 and h == 512
    return np.ascontiguousarray(
        x.reshape(64, 2, 256).transpose(1, 0, 2).reshape(128, 256))


def _host_prep(inputs):
    import ml_dtypes

    bf16 = ml_dtypes.bfloat16
    f32 = np.float32
    feats = np.asarray(inputs["features"], f32)
    caps = np.asarray(inputs["captions"]).astype(np.int64)
    lens = np.asarray(inputs["lengths"]).reshape(-1).astype(np.int64)
    table = np.asarray(inputs["embed_table"], f32)

    W_ih = np.asarray(inputs["W_ih"], f32)
    W_hh = np.asarray(inputs["W_hh"], f32)
    b_ih = np.asarray(inputs["b_ih"], f32)
    b_hh = np.asarray(inputs["b_hh"], f32)
    b_beta = np.asarray(inputs["b_beta"], f32)
    b_fc = np.asarray(inputs["b_fc"], f32)
    b_hinit = np.asarray(inputs["b_hinit"], f32)
    b_cinit = np.asarray(inputs["b_cinit"], f32)

    # ragged-batch packing (lengths sorted descending by construction)
    b_t = [int((lens > t).sum()) for t in range(T)]
    off = np.concatenate([[0], np.cumsum(b_t)]).astype(np.int64)
    p_total = int(off[-1])
    p_pad = ((p_total + 127) // 128) * 128
    mv = p_pad // 128
    t_eff = max(t for t in range(T) if b_t[t] > 0) + 1

    # paired-row bf16 table: super-row index fits int16, one descriptor
    # fetches 2 KiB; parity mask selects the half after the gather.
    VP = V + (V % 2)
    tpad = np.zeros((VP, H), f32)
    tpad[:V] = table
    tableg2 = np.ascontiguousarray(tpad.reshape(VP // 2, 2 * H)).astype(bf16)

    # E/hall rows: row r = off[t]+b.  Rows 0:64 = features (step 0) -> dummy
    # gather idx 0; real gathered rows cover [64, p_total); pad -> idx 0.
    n_gather = p_pad
    idx_flat = np.zeros(n_gather, np.int64)
    for t in range(1, T):
        for b in range(b_t[t]):
            idx_flat[int(off[t]) + b] = caps[b, t - 1]
    idx_sup = (idx_flat // 2).astype(np.int16)
    parity = (idx_flat % 2).astype(f32)

    # gather groups (j = row index), 128-multiples, never splitting a block
    groups = []
    bounds = [0, 128]
    while bounds[-1] < n_gather:
        bounds.append(min(bounds[-1] + 256, n_gather))
    for a, b in zip(bounds[:-1], bounds[1:]):
        groups.append((a, b))
    ipk_meta = []   # (offset, idx_w, n)
    w_tot = 0
    for a, b in groups:
        n = b - a
        ipk_meta.append((w_tot, n // 16, n))
        w_tot += n // 16 + n
    ipk = np.zeros((128, w_tot), np.int16)
    for g, (a, b) in enumerate(groups):
        o, w, n = ipk_meta[g]
        ipk[:, o : o + w] = _pack_idx(idx_sup[a:b])
        mk = np.ascontiguousarray(
            np.tile(parity[a:b][None, :], (128, 1))).astype(bf16)
        ipk[:, o + w : o + w + n] = mk.view(np.int16)

    # weights, folded column order
    w2emb = W_ih.T[:H][:, _CO]                       # [512, 2048]
    w2ah = np.vstack([W_ih.T[H:], W_hh.T])[:, _CO]   # [1024, 2048]

    b2 = (b_ih + b_hh)[_CO]
    has_b2 = bool(np.any(b2))
    has_bbeta = bool(np.any(b_beta))
    has_bfc = bool(np.any(b_fc))
    has_binit = bool(np.any(b_hinit)) or bool(np.any(b_cinit))

    # e_load selectors: selA[k, 64t+m] = 1 iff k == off[t]%128 + m (< 128);
    # selB handles the next-block wrap.
    selA = np.zeros((128, T * 64), f32)
    selB = np.zeros((128, T * 64), f32)
    span = []
    for t in range(T):
        p0 = int(off[t]) % 128
        mb0 = int(off[t]) // 128
        sp = (p0 + 64 > 128) and (mb0 + 1 < mv)
        span.append(bool(sp))
        for m in range(64):
            k = p0 + m
            if k < 128:
                selA[k, 64 * t + m] = 1.0
            elif sp:
                selB[k - 128, 64 * t + m] = 1.0

    common = {
        "tableg2": tableg2,
        "featT": _pack_k(feats.T.astype(f32)).astype(bf16),
        "w2e": _pack_k(w2emb).astype(bf16),
        "w2ah": _pack_k(w2ah).astype(bf16),
        "wbeta": _pack_k(np.asarray(inputs["W_beta"], f32)).astype(bf16),
        "selA": selA.astype(bf16),
        "selB": selB.astype(bf16),
        "idxpack": ipk,
    }
    # constpack: cols 0:128 ident | 128:384 folded features
    cpk = np.zeros((128, 384), bf16)
    cpk[:, 0:128] = np.eye(128, dtype=f32).astype(bf16)
    cpk[:, 128:384] = _fold(feats).astype(bf16)
    common["constpack"] = cpk
    whc = np.zeros((128, KC, 2 * H), bf16)
    whc[:, :, 0:H] = _pack_k(np.asarray(inputs["W_hinit"], f32)).astype(bf16)
    whc[:, :, H : 2 * H] = _pack_k(np.asarray(inputs["W_cinit"], f32)).astype(bf16)
    common["whcpack"] = whc
    if has_b2:
        common["b2rep"] = np.ascontiguousarray(
            np.tile(b2[None, :], (128, 1)).astype(f32))
    if has_bbeta:
        common["bbeta2"] = np.ascontiguousarray(
            np.tile(_fold(np.tile(b_beta[None, :], (64, 1))), (1, 1))).astype(f32)
    if has_binit:
        # hT layout add: bh[k', nh, 64s+b] = b_hinit[256s+128nh+k']
        bh = np.zeros((128, 2, 128), f32)
        for s in range(2):
            for nh in range(2):
                for kp in range(128):
                    bh[kp, nh, 64 * s : 64 * s + 64] = b_hinit[256 * s + 128 * nh + kp]
        common["bhT"] = bh
        common["bc2"] = _fold(np.tile(b_cinit[None, :], (64, 1))).astype(f32)

    W_fc = np.asarray(inputs["W_fc"], f32)
    wfc_pad = np.zeros((H, VPAD), f32)
    wfc_pad[:, :V] = W_fc
    bfc_pad = np.zeros(VPAD, f32)
    bfc_pad[:V] = b_fc

    in_maps = []
    for k in range(NCORES):
        m = dict(common)
        m["wfc"] = _pack_k(wfc_pad[:, k * VS : (k + 1) * VS]).astype(bf16)
        if has_bfc:
            m["bfcrep"] = np.ascontiguousarray(
                np.tile(bfc_pad[k * VS : (k + 1) * VS][None, :], (128, 1))
            ).astype(f32)
        in_maps.append(m)

    meta = {
        "b_t": b_t, "off": [int(x) for x in off], "p_total": p_total,
        "p_pad": p_pad, "t_eff": t_eff, "span": span, "groups": groups,
        "ipk_meta": ipk_meta, "ipk_w": w_tot,
        "has_b2": has_b2, "has_bbeta": has_bbeta, "has_bfc": has_bfc,
        "has_binit": has_binit,
    }
    return in_maps, meta


def build_program(meta):
    b_t = meta["b_t"]
    off = meta["off"]
    p_total = meta["p_total"]
    p_pad = meta["p_pad"]
    t_eff = meta["t_eff"]
    span = meta["span"]
    groups = meta["groups"]
    ipk_meta = meta["ipk_meta"]
    ipk_w = meta["ipk_w"]
    mv = p_pad // 128
    has_b2 = meta["has_b2"]
    has_bbeta = meta["has_bbeta"]
    has_bfc = meta["has_bfc"]
    has_binit = meta["has_binit"]

    nc = bacc.Bacc(num_swdge_queues=1)

    tableg2_d = nc.declare_dram_parameter(
        "tableg2", [(V + (V % 2)) // 2, 2 * H], BF16, isOutput=False)
    featT_d = nc.declare_dram_parameter("featT", [128, KC, B], BF16, isOutput=False)
    w2e_d = nc.declare_dram_parameter("w2e", [128, KC, GATE_N], BF16, isOutput=False)
    w2ah_d = nc.declare_dram_parameter("w2ah", [128, 8, GATE_N], BF16, isOutput=False)
    wbeta_d = nc.declare_dram_parameter("wbeta", [128, KC, H], BF16, isOutput=False)
    constpack_d = nc.declare_dram_parameter("constpack", [128, 384], BF16, isOutput=False)
    whcpack_d = nc.declare_dram_parameter("whcpack", [128, KC, 2 * H], BF16, isOutput=False)
    selA_d = nc.declare_dram_parameter("selA", [128, T * 64], BF16, isOutput=False)
    selB_d = nc.declare_dram_parameter("selB", [128, T * 64], BF16, isOutput=False)
    idxpack_d = nc.declare_dram_parameter("idxpack", [128, ipk_w], I16, isOutput=False)
    wfc_d = nc.declare_dram_parameter("wfc", [128, KC, VS], BF16, isOutput=False)
    if has_b2:
        b2rep_d = nc.declare_dram_parameter("b2rep", [128, GATE_N], F32, isOutput=False)
    if has_bbeta:
        bbeta2_d = nc.declare_dram_parameter("bbeta2", [128, 256], F32, isOutput=False)
    if has_binit:
        bhT_d = nc.declare_dram_parameter("bhT", [128, 2, 128], F32, isOutput=False)
        bc2_d = nc.declare_dram_parameter("bc2", [128, 256], F32, isOutput=False)
    if has_bfc:
        bfcrep_d = nc.declare_dram_parameter("bfcrep", [128, VS], F32, isOutput=False)
    out_d = nc.declare_dram_parameter("out", [p_pad, VS], BF16, isOutput=True)

    def mm(out, lhsT, rhs, start, stop):
        nc.tensor.matmul(out, lhsT, rhs, start=start, stop=stop)

    # vocab item (mc, vo, vw) runnable once all hall rows of block mc are
    # written, i.e. after step mc_ready[mc]'s h transposes.
    mc_ready = []
    for mc in range(mv):
        need = (mc + 1) * 128
        r = t_eff - 1
        for t in range(t_eff):
            if off[t + 1] >= need:
                r = t
                break
        mc_ready.append(r)
    vqueue = [(mc, vo, vw) for mc in range(mv) for (vo, vw) in VCHUNKS]

    # per-step hall write segments: (mc, local_lo, src_lo, n)
    hall_segs = []
    for t in range(T):
        segs = []
        lo, n = off[t], b_t[t]
        while n > 0:
            mc = lo // 128
            ll = lo % 128
            take = min(128 - ll, n)
            segs.append((mc, ll, lo - off[t], take))
            lo += take
            n -= take
        hall_segs.append(segs)

    # E-block needed latest by step t (e_load reads 64 rows from off[t]):
    # block off[t]//128, plus next block when spanning.
    eb_need = [off[t] // 128 + (1 if span[t] else 0) for t in range(t_eff)]

    # gather group covering E row r (groups indexed by row directly)
    def grp_of_row(r):
        for g, (a, b) in enumerate(groups):
            if r < b:
                return g
        return len(groups) - 1

    SIG = mybir.ActivationFunctionType.Sigmoid
    TANH = mybir.ActivationFunctionType.Tanh

    with tile.TileContext(nc) as tc:
        nc.gpsimd.load_library(library_config.mlp)
        with (
            tc.tile_pool(name="const", bufs=1) as constp,
            tc.tile_pool(name="res", bufs=1) as resp,
            tc.tile_pool(name="state", bufs=1) as statep,
            tc.tile_pool(name="step", bufs=1) as sp,
            tc.tile_pool(name="stage", bufs=3) as stp,
            tc.tile_pool(name="gates", bufs=2, space="PSUM") as gps,
            tc.tile_pool(name="tpp", bufs=1, space="PSUM") as tpp,
            tc.tile_pool(name="betap", bufs=1, space="PSUM") as bpp,
            tc.tile_pool(name="fill", bufs=2, space="PSUM") as fillp,
        ):
            # --- small constants first (featT leads: h0/c0 + block-0 E) ---
            featT = constp.tile([128, KC, B], BF16)
            nc.sync.dma_start(featT[:], featT_d[:])
            whc = constp.tile([128, KC, 2 * H], BF16, name="whc")
            nc.sync.dma_start(whc[:], whcpack_d[:])
            cp = constp.tile([128, 384], BF16, name="cp")
            nc.sync.dma_start(cp[:], constpack_d[:])
            selA = constp.tile([128, T * 64], BF16, name="selA")
            nc.sync.dma_start(selA[:], selA_d[:])
            selB = constp.tile([128, T * 64], BF16, name="selB")
            if any(span):
                nc.sync.dma_start(selB[:], selB_d[:])
            ipk = constp.tile([128, ipk_w], I16, name="ipk")
            nc.sync.dma_start(ipk[:], idxpack_d[:])
            ident = cp[:, 0:128]
            feat2 = cp[:, 128:384]
            if has_bbeta:
                bbeta2 = constp.tile([128, 256], F32, name="bbeta2")
                nc.sync.dma_start(bbeta2[:], bbeta2_d[:])
            if has_binit:
                bhT = constp.tile([128, 2, 128], F32, name="bhT")
                nc.sync.dma_start(bhT[:], bhT_d[:])
                bc2 = constp.tile([128, 256], F32, name="bc2")
                nc.sync.dma_start(bc2[:], bc2_d[:])

            # --- persistent residents ---
            w2ah = resp.tile([128, 8, GATE_N], BF16)
            wbeta = resp.tile([128, KC, H], BF16)
            e_sb = resp.tile([128, mv, 4, 512], BF16, name="E")
            hall_t = [
                resp.tile([128, KC, 128], BF16, tag=f"hall{mc}", name=f"hall{mc}")
                for mc in range(mv)
            ]
            if p_pad > p_total:
                mc = p_total // 128
                nc.vector.memset(hall_t[mc][:, :, p_total % 128 :], 0.0)
                for m2 in range(mc + 1, mv):
                    nc.vector.memset(hall_t[m2][:], 0.0)
            if has_b2:
                b2rep = resp.tile([128, GATE_N], F32)
            if has_bfc:
                bfcrep = resp.tile([128, VS], F32)
            wfc = resp.tile([128, KC, VS], BF16)

            hT = statep.tile([128, 2, 128], BF16)
            aweT = statep.tile([128, 2, 128], BF16)
            c2 = statep.tile([128, 256], F32)
            h2 = statep.tile([128, 256], BF16)

            # ============ helpers ============
            vstate = {"q": 0, "alt": 0}

            def vocab_item():
                mc, vo, vw = vqueue[vstate["q"]]
                vstate["q"] += 1
                pv = fillp.tile([128, 512], F32, tag="fill")
                for kc in range(KC):
                    mm(pv[:, :vw], hall_t[mc][:, kc, :], wfc[:, kc, vo : vo + vw],
                       start=(kc == 0), stop=(kc == KC - 1))
                st = stp.tile([128, 512], BF16, tag="st")
                if has_bfc:
                    nc.vector.tensor_add(st[:, :vw], pv[:, :vw],
                                         bfcrep[:, vo : vo + vw])
                elif vstate["alt"] == 0:
                    nc.scalar.copy(st[:, :vw], pv[:, :vw])
                else:
                    nc.vector.tensor_copy(st[:, :vw], pv[:, :vw])
                vstate["alt"] ^= 1
                nc.sync.dma_start(
                    out_d[mc * 128 : (mc + 1) * 128, vo : vo + vw], st[:, :vw])

            def emit_vocab(t, cap):
                n = 0
                while (vstate["q"] < len(vqueue) and n < cap
                       and mc_ready[vqueue[vstate["q"]][0]] < t):
                    vocab_item()
                    n += 1

            def hT_slice(src, kc):
                # contraction chunk kc -> hT/aweT [:, kc%2, 64*(kc//2):+64]
                s, nh = kc // 2, kc % 2
                return src[:, nh, 64 * s : 64 * s + 64]

            def e_load(t, gfi, ggo):
                sA = selA[:, 64 * t : 64 * t + 64]
                for tile_, cbl in ((gfi, 0), (ggo, 2)):
                    mb = off[t] // 128
                    mm(tile_[0:64, :], sA, e_sb[:, mb, cbl, :],
                       start=True, stop=False)
                    mm(tile_[64:128, :], sA, e_sb[:, mb, cbl + 1, :],
                       start=True, stop=False)
                    if span[t]:
                        sB = selB[:, 64 * t : 64 * t + 64]
                        mm(tile_[0:64, :], sB, e_sb[:, mb + 1, cbl, :],
                           start=False, stop=False)
                        mm(tile_[64:128, :], sB, e_sb[:, mb + 1, cbl + 1, :],
                           start=False, stop=False)

            def h_trans(t, hsrc):
                """Transpose folded h (prev step's output) into hT + hall."""
                tph = tpp.tile([128, 256], BF16, tag="tp")
                nc.tensor.transpose(tph[:, 0:128], hsrc[:, 0:128], ident)
                nc.tensor.transpose(tph[:, 128:256], hsrc[:, 128:256], ident)
                for nh in range(2):
                    nc.vector.tensor_copy(hT[:, nh, :],
                                          tph[:, 128 * nh : 128 * nh + 128])
                for kc in range(KC):
                    s, nh = kc // 2, kc % 2
                    for (mc, ll, sl, seg_n) in hall_segs[t]:
                        nc.vector.tensor_copy(
                            hall_t[mc][:, kc, ll : ll + seg_n],
                            tph[:, 128 * nh + 64 * s + sl :
                                   128 * nh + 64 * s + sl + seg_n])

            def beta_mms():
                bp = bpp.tile([128, 256], F32, tag="beta")
                for kc in range(KC):
                    lhs = hT_slice(hT, kc)
                    mm(bp[0:64, :], lhs, wbeta[:, kc, 0:256],
                       start=(kc == 0), stop=(kc == KC - 1))
                    mm(bp[64:128, :], lhs, wbeta[:, kc, 256:512],
                       start=(kc == 0), stop=(kc == KC - 1))
                if has_bbeta:
                    nc.vector.tensor_add(bp[:], bp[:], bbeta2[:])
                return bp

            def h_part(gfi, ggo):
                for tile_, cbl in ((gfi, 0), (ggo, 2)):
                    for kc in range(KC):
                        lhs = hT_slice(hT, kc)
                        mm(tile_[0:64, :], lhs, w2ah[:, 4 + kc, cbl * 512 :
                                                     cbl * 512 + 512],
                           start=False, stop=False)
                        mm(tile_[64:128, :], lhs, w2ah[:, 4 + kc, (cbl + 1) * 512 :
                                                       (cbl + 1) * 512 + 512],
                           start=False, stop=False)

            def sig_awe(bp):
                sigb2 = sp.tile([128, 256], BF16, tag="sigb")
                nc.scalar.activation(sigb2[:], bp[:], SIG)
                awe2 = sp.tile([128, 256], BF16, tag="awe2")
                nc.vector.tensor_mul(awe2[:], sigb2[:], feat2)
                tpa = tpp.tile([128, 256], BF16, tag="tp")
                nc.tensor.transpose(tpa[:, 0:128], awe2[:, 0:128], ident)
                nc.tensor.transpose(tpa[:, 128:256], awe2[:, 128:256], ident)
                for nh in range(2):
                    nc.vector.tensor_copy(aweT[:, nh, :],
                                          tpa[:, 128 * nh : 128 * nh + 128])

            def awe_mms(gfi, ggo):
                # fi tile first so sig_fi starts while go streams
                for tile_, cbl in ((gfi, 0), (ggo, 2)):
                    for kc in range(KC):
                        lhs = hT_slice(aweT, kc)
                        mm(tile_[0:64, :], lhs, w2ah[:, kc, cbl * 512 :
                                                     cbl * 512 + 512],
                           start=False, stop=(kc == KC - 1))
                        mm(tile_[64:128, :], lhs, w2ah[:, kc, (cbl + 1) * 512 :
                                                       (cbl + 1) * 512 + 512],
                           start=False, stop=(kc == KC - 1))

            def pointwise(gfi, ggo):
                sigfi = sp.tile([128, 512], F32, tag="sigfi")
                nc.scalar.activation(sigfi[:], gfi[:], SIG)
                nc.vector.tensor_mul(c2[:], c2[:], sigfi[:, 0:256])
                tanhg = sp.tile([128, 256], F32, tag="tanhg")
                nc.scalar.activation(tanhg[:], ggo[:, 0:256], TANH)
                t2 = sp.tile([128, 256], F32, tag="t2")
                nc.vector.tensor_mul(t2[:], sigfi[:, 256:512], tanhg[:])
                nc.vector.tensor_add(c2[:], c2[:], t2[:])
                sigo = sp.tile([128, 256], BF16, tag="sigo")
                nc.scalar.activation(sigo[:], ggo[:, 256:512], SIG)
                tanhc = sp.tile([128, 256], BF16, tag="tanhc")
                nc.scalar.activation(tanhc[:], c2[:], TANH)
                nc.vector.tensor_mul(h2[:], sigo[:], tanhc[:])

            # ============ prep phase ============
            with (
                tc.tile_pool(name="prew", bufs=1) as prew,
                tc.tile_pool(name="gath", bufs=2) as gp,
            ):
                glo = [None] * len(groups)
                maskt = [None] * len(groups)

                def gather(g):
                    a, b = groups[g]
                    n = b - a
                    o, w, _ = ipk_meta[g]
                    maskt[g] = ipk[:, o + w : o + w + n].bitcast(BF16)
                    glo[g] = gp.tile([128, 2 * KC, n], BF16, tag=f"graw{g}",
                                     bufs=1, name=f"graw{g}")
                    nc.gpsimd.dma_gather(
                        glo[g][:], tableg2_d[:],
                        ipk[:, o : o + w], num_idxs=n, num_idxs_reg=n,
                        elem_size=2 * H, transpose=True, queue_num=0,
                    )

                def blend(g):
                    a, b = groups[g]
                    n = b - a
                    for c in range(KC):
                        dtile = gp.tile([128, 256], BF16, tag="gd", bufs=2,
                                        name="gd")
                        nc.vector.tensor_sub(
                            dtile[:, :n], glo[g][:, KC + c, :], glo[g][:, c, :])
                        nc.vector.tensor_mul(dtile[:, :n], dtile[:, :n], maskt[g])
                        nc.vector.tensor_add(
                            glo[g][:, c, :], glo[g][:, c, :], dtile[:, :n])

                for g in range(len(groups)):
                    gather(g)

                # warm the sigmoid/tanh ACT table during the DMA wait
                warm = sp.tile([1, 2], F32, tag="warm")
                nc.scalar.activation(warm[:], cp[0:1, 0:2], SIG)

                # weight DMAs ordered by first use; wfc split in two
                nc.scalar.dma_start(wbeta[:], wbeta_d[:])
                w2e = prew.tile([128, KC, GATE_N], BF16, bufs=1)
                nc.scalar.dma_start(w2e[:], w2e_d[:])
                nc.scalar.dma_start(w2ah[:, 4:8, :], w2ah_d[:, 4:8, :])
                nc.scalar.dma_start(w2ah[:, 0:4, :], w2ah_d[:, 0:4, :])
                if has_b2:
                    nc.scalar.dma_start(b2rep[:], b2rep_d[:])
                if has_bfc:
                    nc.scalar.dma_start(bfcrep[:], bfcrep_d[:])
                nc.scalar.dma_start(wfc[:, :, 0:3072], wfc_d[:, :, 0:3072])
                nc.scalar.dma_start(wfc[:, :, 3072:VS], wfc_d[:, :, 3072:VS])

                # h0 (transposed directly into hT) and c0 (folded)
                for jb in range(KC):
                    hps = fillp.tile([128, 512], F32, tag="fill")
                    for kc in range(KC):
                        mm(hps[:, 0:B], whc[:, kc, jb * 128 : (jb + 1) * 128],
                           featT[:, kc, :], start=(kc == 0), stop=(kc == KC - 1))
                    s, nh = jb // 2, jb % 2
                    dst = hT[:, nh, 64 * s : 64 * s + 64]
                    if has_binit:
                        nc.vector.tensor_add(dst, hps[:, 0:B],
                                             bhT[:, nh, 64 * s : 64 * s + 64])
                    else:
                        nc.vector.tensor_copy(dst, hps[:, 0:B])
                cps = fillp.tile([128, 512], F32, tag="fill")
                for kc in range(KC):
                    lhs = featT[:, kc, :]
                    mm(cps[0:64, 0:256], lhs, whc[:, kc, H : H + 256],
                       start=(kc == 0), stop=(kc == KC - 1))
                    mm(cps[64:128, 0:256], lhs, whc[:, kc, H + 256 : H + 512],
                       start=(kc == 0), stop=(kc == KC - 1))
                if has_binit:
                    nc.vector.tensor_add(c2[:], cps[:, 0:256], bc2[:])
                else:
                    nc.vector.tensor_copy(c2[:], cps[:, 0:256])

                # ---- E-prep ----
                ep_alt = [0]

                def e_copy(dst, src, cb):
                    if has_b2:
                        # b2 folded into E so e_load carries the bias
                        nc.vector.tensor_add(dst, src,
                                             b2rep[:, cb * 512 : (cb + 1) * 512])
                    elif ep_alt[0] == 0:
                        nc.scalar.copy(dst, src)
                    else:
                        nc.vector.tensor_copy(dst, src)
                    ep_alt[0] ^= 1

                def e_pre0():
                    """Block 0: rows 0:64 features (T0), 64:128 gather g0 (T1)."""
                    for cb in range(4):
                        pse = fillp.tile([128, 512], F32, tag="fill")
                        for kc in range(KC):
                            mm(pse[0:64, :], featT[:, kc, :],
                               w2e[:, kc, cb * 512 : (cb + 1) * 512],
                               start=(kc == 0), stop=(kc == KC - 1))
                            mm(pse[64:128, :], glo[0][:, kc, 64:128],
                               w2e[:, kc, cb * 512 : (cb + 1) * 512],
                               start=(kc == 0), stop=(kc == KC - 1))
                        e_copy(e_sb[:, 0, cb, :], pse[:], cb)

                def e_pre(mb):
                    """Full gathered block mb (rows 128mb..128mb+128)."""
                    g = grp_of_row(128 * mb)
                    a, b = groups[g]
                    c0 = 128 * mb - a
                    for cb in range(4):
                        pse = fillp.tile([128, 512], F32, tag="fill")
                        for kc in range(KC):
                            mm(pse[:, :], glo[g][:, kc, c0 : c0 + 128],
                               w2e[:, kc, cb * 512 : (cb + 1) * 512],
                               start=(kc == 0), stop=(kc == KC - 1))
                        e_copy(e_sb[:, mb, cb, :], pse[:], cb)

                # ---- steps ----
                def step(t):
                    gfi = gps.tile([128, 512], F32, tag="gfi", name="gfi")
                    ggo = gps.tile([128, 512], F32, tag="ggo", name="ggo")
                    e_load(t, gfi, ggo)
                    if t > 0:
                        h_trans(t - 1, h2)
                    bp = beta_mms()
                    h_part(gfi, ggo)
                    sig_awe(bp)
                    awe_mms(gfi, ggo)
                    pointwise(gfi, ggo)
                    emit_vocab(t, 4 if t < t_eff - 6 else 100)

                # interleave: block-0 E, then steps, with e_pre(mb) emitted
                # one step before the first step that e_loads it.
                blend(0)
                e_pre0()
                blended = {0}
                state_mb = {"next": 1, "tgt": 0}
                for t in range(t_eff):
                    state_mb["tgt"] = max(state_mb["tgt"],
                                          eb_need[min(t + 1, t_eff - 1)])
                    while state_mb["next"] <= min(state_mb["tgt"], mv - 1):
                        mb = state_mb["next"]
                        g = grp_of_row(128 * mb)
                        if g not in blended:
                            blend(g)
                            blended.add(g)
                        e_pre(mb)
                        state_mb["next"] += 1
                    step(t)
                # remaining blocks (pad rows, never read by e_load) skipped

            h_trans(t_eff - 1, h2)
            while vstate["q"] < len(vqueue):
                vocab_item()

    nc.finalize()
    return nc


def kernel(**inputs):
    in_maps, meta = _host_prep(inputs)
    nc = build_program(meta)
    res = run_bass_kernel_spmd(nc, in_maps, core_ids=list(range(NCORES)))
    results = res.results

    b_t = meta["b_t"]
    off = meta["off"]
    full = np.zeros((B, T, VPAD), np.float32)
    for k in range(NCORES):
        o = np.asarray(results[k]["out"]).astype(np.float32)
        for t in range(T):
            bt = b_t[t]
            if bt:
                full[:bt, t, k * VS : (k + 1) * VS] = o[off[t] : off[t] + bt]
    return full[:, :, :V]
